# revision 3
# baseline (speedup 1.0000x reference)
"""Trainium2 Bass kernel for BrainFunctionalConnectivityFeatureExtractionModule.

Math (per batch b, all f32):
    w    = relu(adj + adj_bias)                       (16,16)
    d    = 1/sqrt(sum(w, axis=1) + 1e-5)              (16,)
    lap  = I - d[:,None] * w * d[None,:]              (16,16)
    t1   = lap @ x[b]                                 (16,256)
    h    = relu(bias_h + t1 @ cheb_w[1::2])           (16,64)
    out  = h @ fc_w.T + fc_b                          (16,387)
(bias_h folds the all-ones Chebyshev-T0 lanes; fc_b is added on the host,
fused into the f32 upcast of the output, freeing stage 3 to use a clean
K=64 contraction that packs 2-per-PE-array.)

Data parallel over 8 cores; 16384 (b,e)-rows/core in 32 macro tiles of
512 rows (4 x 128-row sub-tiles, sub-tile = 8 full 16-node graphs).

Per-tile dataflow (PE contraction must run along SBUF partitions, and this
is the unique transpose-free chaining of the three matmuls):
  stage 1  t1T[c, n] = x_sub.T @ (I_8 (x) lap^T)     8 MMs N=128
  casts    t1 PSUM f32 -> SBUF bf16                  (DVE, 2 ops)
  stage 2  hT, COL-PACKED: the two 256-col n-halves of the tile run on
           PE column-groups 0-1/2-3 concurrently (auto tile_position via
           out base_partition 0/64), so h lands as [128=(nhalf,h), 256]
           in half the cycles of the unpacked M=64 matmul.
  relu     Act activation w/ per-partition bias (duplicated per n-half),
           FIRST in the Act stream each iteration, so the single h bank
           is always drained before the next stage 2 writes it.
  stage 3  out[128, 388] = hT_slice.T @ fcwT, ROW-PACKED pairs: the K=64
           contraction for n-half 0 lives in PE row-groups 0-1 (SBUF
           partitions 0-63) and for n-half 1 in row-groups 2-3
           (partitions 64-127) -- exactly where col-packed stage 2 put
           them.  fcwT is duplicated on both partition halves.
  copies   out PSUM f32 -> SBUF bf16: chunks 0-1 merged as one Act
           pair-copy, chunks 2 and 3 as singles on Act and DVE; then
           one bf16 DMA store.

Software pipeline, 3-tile lag -- iteration it runs:
  PE:     s1k0(it), s3(it-3), s2(it-1) issued twice (heat), s1k1(it)
  DVE:    relu(it-2), cast_k0(it-1), cast_k1(it-1)
  Act:    out pair-copy A (it-3), out singles B0/B1 (it-3) -- all
          Identity, so the activation table never switches
  sync:   x-load(it)          (loads alone on the SP HWDGE queue)
  gpsimd: out-store(it-3) via SWDGE (a store sem-wait on the sync queue
          would head-of-line block the next x prefetch)
PSUM (8 banks): t1 k0 double-buffered pool tile + k1 single (3), h (1),
out pairs A/B as two 2-bank pool tiles (4), so the next tile's pair-A
matmuls wait only on the early Act pair-copy.

PE clock: PE_HAM leaves the PE clock-gated at K=4/8 (1.2 GHz) until it
sees ~3.4us of sustained activity, and re-throttles whenever the duty
cycle drops; measured: without intervention this kernel runs every
matmul at half rate.  The preamble issues 8 dummy back-to-back N=512
matmuls (on a DVE-memset SBUF tile) in the shadow of the initial
weight/x DMAs to release the gate, and stage 2's matmul group is
issued twice per tile (identical result) as dependency-free heat to
keep the duty cycle above the re-throttle threshold.  Residual HAM
oscillation remains the main inefficiency at full scale.

All device I/O is bf16 (the kernel is HBM-heavy): x cast on host, out
stored bf16 and upcast (+fc_b) on host.  Measured rel-err ~3.6e-3 vs
the 2e-2 gate.
"""

import numpy as np
from contextlib import ExitStack

B, E, C, H, OUT = 8192, 16, 256, 64, 387
NCORES = 8
ROWS = (B // NCORES) * E        # 16384 rows per core
NS = 4                          # sub-tiles per macro tile
TR = 128 * NS                   # 512 macro-tile rows
NT = ROWS // TR                 # 32 macro tiles per core
KC = C // 128                   # 2 contraction chunks of 128
OUTP = OUT + 1                  # fc matmul N padded even
HH = TR // 2                    # n-half size (256)

_cache = {}


def _build_module(nt=NT):
    import concourse.tile as tile
    from concourse import bacc, mybir

    f32 = mybir.dt.float32
    bf16 = mybir.dt.bfloat16
    Relu = mybir.ActivationFunctionType.Relu

    nc = bacc.Bacc("TRN2", target_bir_lowering=False, debug=False,
                   num_devices=NCORES)

    rows = nt * TR
    x_d = nc.dram_tensor("x", (rows, C), bf16, kind="ExternalInput").ap()
    r_d = nc.dram_tensor("r", (128, 128), bf16, kind="ExternalInput").ap()
    w1_d = nc.dram_tensor("w1", (KC, 128, H), bf16, kind="ExternalInput").ap()
    bh_d = nc.dram_tensor("bh", (128, 1), f32, kind="ExternalInput").ap()
    fcw_d = nc.dram_tensor("fcw", (128, OUTP), bf16, kind="ExternalInput").ap()
    o_d = nc.dram_tensor("o", (rows, OUT), bf16, kind="ExternalOutput").ap()

    with tile.TileContext(nc) as tc:
        with ExitStack() as ctx:
            consts = ctx.enter_context(tc.tile_pool(name="consts", bufs=1))
            xp = ctx.enter_context(tc.tile_pool(name="xp", bufs=6))
            t1sp = ctx.enter_context(tc.tile_pool(name="t1sp", bufs=3))
            hp = ctx.enter_context(tc.tile_pool(name="hp", bufs=3))
            op = ctx.enter_context(tc.tile_pool(name="op", bufs=3))
            t1pp = ctx.enter_context(tc.tile_pool(name="t1pp", bufs=1, space="PSUM"))
            hpp = ctx.enter_context(tc.tile_pool(name="hpp", bufs=1, space="PSUM"))
            opp = ctx.enter_context(tc.tile_pool(name="opp", bufs=1, space="PSUM"))

            # weights go on the Act DGE queue so the SP queue's head can
            # feed tile-0 x immediately
            r_sb = consts.tile([128, 128], bf16)
            nc.scalar.dma_start(r_sb, r_d)
            w1_sb = consts.tile([128, KC, H], bf16)
            nc.scalar.dma_start(w1_sb, w1_d.rearrange("k p h -> p k h"))
            bh_sb = consts.tile([128, 1], f32)
            nc.scalar.dma_start(bh_sb, bh_d)
            fcw_sb = consts.tile([128, OUTP], bf16)
            nc.scalar.dma_start(fcw_sb, fcw_d)

            # single h bank; stage 2 waits for relu's read each
            # iteration, which runs first in the Act stream
            h_ps = hpp.tile([128, HH], f32)

            # HAM warm-up: 4 dummy back-to-back matmuls during the DMA
            # preamble shadow start releasing the PE clock gate (1.2 ->
            # 2.4 GHz); the first real (cold) tiles continue the heating,
            # so the burst only needs to bridge until tile-0 compute
            # starts, not supply the full ~3.4us itself
            warm_sb = consts.tile([128, 512], bf16, name="warm")
            nc.vector.memset(warm_sb, 0.0)
            warm_ps = opp.tile([128, 2, 512], f32, name="oA", bufs=1)
            for _ in range(4):
                nc.tensor.matmul(warm_ps[:, 0, :], lhsT=warm_sb[:, 0:128],
                                 rhs=warm_sb, skip_group_check=True)

            # x: row l of macro t lives at sub-tile l//128, partition l%128
            xv = x_d.rearrange("(t s p) c -> t p s c", p=128, s=NS)
            # out: row l = 256*X + 2p + s  ->  o_sb[p, X, s, :]
            ov = o_d.rearrange("(t x p s) o -> t p x s o", p=128, s=2, x=2)

            xq, t1q, k0q, k1q, hbq, oq = {}, {}, {}, {}, {}, {}
            for it in range(nt + 3):
                j, f, g = it - 1, it - 2, it - 3

                if it < nt:
                    t1k0_ps = t1pp.tile([128, TR], f32, name="t1k0", bufs=2)
                    # stage 1 (k=0 half): t1T[c, n] = x[:,s,c0].T @ (I8 (x) lapT)
                    x_sb = xp.tile([128, NS, C], bf16)
                    nc.sync.dma_start(x_sb, xv[it])
                    xq[it] = x_sb
                    for s in range(NS):
                        nc.tensor.matmul(
                            t1k0_ps[:, s * 128:(s + 1) * 128],
                            lhsT=x_sb[:, s, 0:128],
                            rhs=r_sb,
                        )
                    k0q[it] = t1k0_ps

                if 0 <= g < nt:
                    # stage 3: row-packed pairs; pair s -> 2-bank pool
                    # tile oA (s=0) / oB (s=1), chunk X.  Separate pool
                    # tiles so next tile's pair-A matmuls wait only on
                    # the (early) Act pair-copy, not the late DVE single.
                    hb = hbq.pop(g)
                    hT_v = hb.rearrange("p (n s) -> p s n", s=2)
                    oA_ps = opp.tile([128, 2, 512], f32, name="oA", bufs=1)
                    oB_ps = opp.tile([128, 2, 512], f32, name="oB", bufs=1)
                    for s, ops in ((0, oA_ps), (1, oB_ps)):
                        nc.tensor.matmul(ops[:, 0, 0:OUTP],
                                         lhsT=hT_v[0:64, s, :],
                                         rhs=fcw_sb[0:64, :])
                        nc.tensor.matmul(ops[:, 1, 0:OUTP],
                                         lhsT=hT_v[64:128, s, :],
                                         rhs=fcw_sb[64:128, :])
                    oq[g] = (oA_ps, oB_ps)

                if 0 <= f < nt:
                    # relu+bias on DVE (max(h + bias, 0)), first in its
                    # stream: the h bank is drained before this
                    # iteration's s2 writes it, and the Act stream stays
                    # all-Identity (no activation-table switching)
                    hb = hp.tile([128, HH], bf16)
                    nc.vector.tensor_scalar(hb, h_ps, bh_sb, 0.0,
                                            mybir.AluOpType.add,
                                            mybir.AluOpType.max)
                    hbq[f] = hb

                if 0 <= j < nt:
                    # t1 casts PSUM f32 -> SBUF bf16 on DVE.  Separate
                    # k0/k1 ops: s2's first pair unblocks on k0 alone.
                    t1_sb = t1sp.tile([128, KC, TR], bf16)
                    nc.vector.tensor_copy(t1_sb[:, 0, :], k0q.pop(j))
                    nc.vector.tensor_copy(t1_sb[:, 1, :], k1q.pop(j))
                    t1q[j] = t1_sb

                if 0 <= g < nt:
                    # out copies PSUM f32 -> SBUF bf16: pair A as one Act
                    # pair-copy, pair B split: chunk 0 on Act, 1 on DVE
                    oA_ps, oB_ps = oq.pop(g)
                    o_sb = op.tile([128, 2, 2, OUT], bf16)
                    nc.scalar.copy(o_sb[:, :, 0, :], oA_ps[:, :, 0:OUT])
                    nc.scalar.copy(o_sb[:, 0, 1, :], oB_ps[:, 0, 0:OUT])
                    nc.scalar.copy(o_sb[:, 1, 1, :], oB_ps[:, 1, 0:OUT])
                    # store via the (otherwise idle) gpsimd SWDGE queue:
                    # sharing the sync HWDGE queue with the x-loads lets
                    # the store's semaphore wait head-of-line block the
                    # next x prefetch, starving stage 1
                    nc.gpsimd.dma_start(ov[g], o_sb)

                if 0 <= j < nt:
                    # stage 2, col-packed n-halves.  The group is issued
                    # TWICE (identical result; start=True replays the
                    # accumulation) -- pure PE heat to hold the duty
                    # cycle above PE_HAM's re-throttle threshold without
                    # adding any cross-engine dependency.
                    t1_sb = t1q.pop(j)
                    for k in list(range(KC)) * 2:
                        nc.tensor.matmul(
                            h_ps[0:64, :],
                            lhsT=w1_sb[:, k, :],
                            rhs=t1_sb[:, k, 0:HH],
                            start=(k == 0),
                            stop=(k == KC - 1),
                            skip_group_check=True,
                        )
                        nc.tensor.matmul(
                            h_ps[64:128, :],
                            lhsT=w1_sb[:, k, :],
                            rhs=t1_sb[:, k, HH:TR],
                            start=(k == 0),
                            stop=(k == KC - 1),
                            skip_group_check=True,
                        )

                if it < nt:
                    # stage 1 (k=1 half) last; its cast (2nd DVE op next
                    # iteration) finishes just before the next k1 write
                    x_sb = xq.pop(it)
                    t1k1_ps = t1pp.tile([128, TR], f32, name="t1k1", bufs=1)
                    for s in range(NS):
                        nc.tensor.matmul(
                            t1k1_ps[:, s * 128:(s + 1) * 128],
                            lhsT=x_sb[:, s, 128:256],
                            rhs=r_sb,
                        )
                    k1q[it] = t1k1_ps

    nc.finalize()
    return nc


def _host_prep(adj, adj_bias, cheb_w, brelu_bias, fc_w, fc_b):
    import ml_dtypes

    bf = ml_dtypes.bfloat16
    adj = np.asarray(adj, np.float32)
    w = np.maximum(adj + np.float32(adj_bias.reshape(())), 0.0)
    d = 1.0 / np.sqrt(w.sum(axis=1) + np.float32(1e-5))
    lap = np.eye(E, dtype=np.float32) - d[:, None] * w * d[None, :]

    # r = I_8 (x) lap^T : [p = b*16+j, n = b*16+i] -> lap[i, j]
    r = np.kron(np.eye(128 // E, dtype=np.float32), lap.T)

    cheb_w = np.asarray(cheb_w, np.float32)
    w1 = np.ascontiguousarray(cheb_w[1::2, :]).reshape(KC, 128, H)
    bias_h = (cheb_w[0::2, :].sum(axis=0)
              + np.asarray(brelu_bias, np.float32).reshape(H))
    bh2 = np.concatenate([bias_h, bias_h]).reshape(128, 1)
    fcw = np.zeros((128, OUTP), np.float32)
    fcw[0:H, :OUT] = np.asarray(fc_w, np.float32).T
    fcw[H:128, :OUT] = np.asarray(fc_w, np.float32).T
    return {
        "r": r.astype(bf),
        "w1": np.ascontiguousarray(w1).astype(bf),
        "bh": bh2.astype(np.float32),
        "fcw": fcw.astype(bf),
    }


def _run(inputs, trace=False, nt=NT, **kw):
    import ml_dtypes
    from concourse import bass_utils

    if nt not in _cache:
        _cache[nt] = _build_module(nt=nt)
    nc = _cache[nt]

    x = np.asarray(inputs["x"], np.float32).astype(ml_dtypes.bfloat16)
    weights = _host_prep(inputs["adj"], inputs["adj_bias"], inputs["cheb_w"],
                         inputs["brelu_bias"], inputs["fc_w"], inputs["fc_b"])

    rows = nt * TR
    shards = x.reshape(NCORES, ROWS, C)[:, :rows]
    in_maps = [dict(weights, x=np.ascontiguousarray(shards[c]))
               for c in range(NCORES)]

    res = bass_utils.run_bass_kernel_spmd(
        nc, in_maps, core_ids=list(range(NCORES)), trace=trace, **kw)

    fc_b = np.asarray(inputs["fc_b"], np.float32)
    out = np.concatenate(
        [np.asarray(res.results[c]["o"], dtype=np.float32)
           .reshape(rows // E, E, OUT)
         for c in range(NCORES)], axis=0)
    out += fc_b
    return out, res


def kernel(**inputs) -> np.ndarray:
    out, _ = _run(inputs, trace=False)
    return out


# revision 4
# speedup vs baseline: 1.0702x; 1.0702x over previous
"""Trainium2 Bass kernel for BrainFunctionalConnectivityFeatureExtractionModule.

Math (per batch b, all f32):
    w    = relu(adj + adj_bias)                       (16,16)
    d    = 1/sqrt(sum(w, axis=1) + 1e-5)              (16,)
    lap  = I - d[:,None] * w * d[None,:]              (16,16)
    t1   = lap @ x[b]                                 (16,256)
    h    = relu(bias_h + t1 @ cheb_w[1::2])           (16,64)
    out  = h @ fc_w.T + fc_b                          (16,387)
(bias_h folds the all-ones Chebyshev-T0 lanes; fc_b is added on the host,
fused into the f32 upcast of the output, freeing stage 3 to use a clean
K=64 contraction that packs 2-per-PE-array.)

Data parallel over 8 cores; 16384 (b,e)-rows/core in 32 macro tiles of
512 rows (4 x 128-row sub-tiles, sub-tile = 8 full 16-node graphs).

Per-tile dataflow (PE contraction must run along SBUF partitions, and this
is the unique transpose-free chaining of the three matmuls):
  stage 1  t1T[c, n] = x_sub.T @ (I_8 (x) lap^T)     8 MMs N=128
  casts    t1 PSUM f32 -> SBUF bf16                  (DVE, 2 ops)
  stage 2  hT, COL-PACKED: the two 256-col n-halves of the tile run on
           PE column-groups 0-1/2-3 concurrently (auto tile_position via
           out base_partition 0/64), so h lands as [128=(nhalf,h), 256]
           in half the cycles of the unpacked M=64 matmul.
  relu     Act activation w/ per-partition bias (duplicated per n-half),
           FIRST in the Act stream each iteration, so the single h bank
           is always drained before the next stage 2 writes it.
  stage 3  out[128, 388] = hT_slice.T @ fcwT, ROW-PACKED pairs: the K=64
           contraction for n-half 0 lives in PE row-groups 0-1 (SBUF
           partitions 0-63) and for n-half 1 in row-groups 2-3
           (partitions 64-127) -- exactly where col-packed stage 2 put
           them.  fcwT is duplicated on both partition halves.
  copies   out PSUM f32 -> SBUF bf16: chunks 0-1 merged as one Act
           pair-copy, chunks 2 and 3 as singles on Act and DVE; then
           one bf16 DMA store.

Software pipeline, 3-tile lag -- iteration it runs:
  PE:     s1k0(it), s3(it-3), s2(it-1) issued twice (heat), s1k1(it)
  DVE:    relu(it-2), cast_k0(it-1), cast_k1(it-1)
  Act:    out pair-copy A (it-3), out singles B0/B1 (it-3) -- all
          Identity, so the activation table never switches
  sync:   x-load(it)          (loads alone on the SP HWDGE queue)
  gpsimd: out-store(it-3) via SWDGE (a store sem-wait on the sync queue
          would head-of-line block the next x prefetch)
PSUM (8 banks): t1 k0 double-buffered pool tile + k1 single (3), h (1),
out pairs A/B as two 2-bank pool tiles (4), so the next tile's pair-A
matmuls wait only on the early Act pair-copy.

PE clock: PE_HAM leaves the PE clock-gated at K=4/8 (1.2 GHz) until it
sees ~3.4us of sustained activity, and re-throttles whenever the duty
cycle drops; measured: without intervention this kernel runs every
matmul at half rate.  The preamble issues 8 dummy back-to-back N=512
matmuls (on a DVE-memset SBUF tile) in the shadow of the initial
weight/x DMAs to release the gate, and stage 2's matmul group is
issued twice per tile (identical result) as dependency-free heat to
keep the duty cycle above the re-throttle threshold.  Residual HAM
oscillation remains the main inefficiency at full scale.

All device I/O is bf16 (the kernel is HBM-heavy): x cast on host, out
stored bf16 and upcast (+fc_b) on host.  Measured rel-err ~3.6e-3 vs
the 2e-2 gate.
"""

import numpy as np
from contextlib import ExitStack

B, E, C, H, OUT = 8192, 16, 256, 64, 387
NCORES = 8
ROWS = (B // NCORES) * E        # 16384 rows per core
NS = 4                          # sub-tiles per macro tile
TR = 128 * NS                   # 512 macro-tile rows
NT = ROWS // TR                 # 32 macro tiles per core
KC = C // 128                   # 2 contraction chunks of 128
OUTP = OUT + 1                  # fc matmul N padded even
HH = TR // 2                    # n-half size (256)

_cache = {}


def _build_module(nt=NT):
    assert nt % 2 == 0, "double-tile x loads require even nt"
    import concourse.tile as tile
    from concourse import bacc, mybir

    f32 = mybir.dt.float32
    bf16 = mybir.dt.bfloat16
    Relu = mybir.ActivationFunctionType.Relu

    nc = bacc.Bacc("TRN2", target_bir_lowering=False, debug=False,
                   num_devices=NCORES)

    rows = nt * TR
    x_d = nc.dram_tensor("x", (rows, C), bf16, kind="ExternalInput").ap()
    r_d = nc.dram_tensor("r", (128, 128), bf16, kind="ExternalInput").ap()
    w1_d = nc.dram_tensor("w1", (KC, 128, H), bf16, kind="ExternalInput").ap()
    bh_d = nc.dram_tensor("bh", (128, 1), f32, kind="ExternalInput").ap()
    fcw_d = nc.dram_tensor("fcw", (128, OUTP), bf16, kind="ExternalInput").ap()
    o_d = nc.dram_tensor("o", (rows, OUT), bf16, kind="ExternalOutput").ap()

    with tile.TileContext(nc) as tc:
        with ExitStack() as ctx:
            consts = ctx.enter_context(tc.tile_pool(name="consts", bufs=1))
            xp = ctx.enter_context(tc.tile_pool(name="xp", bufs=6))
            t1sp = ctx.enter_context(tc.tile_pool(name="t1sp", bufs=3))
            hp = ctx.enter_context(tc.tile_pool(name="hp", bufs=3))
            op = ctx.enter_context(tc.tile_pool(name="op", bufs=3))
            t1pp = ctx.enter_context(tc.tile_pool(name="t1pp", bufs=1, space="PSUM"))
            hpp = ctx.enter_context(tc.tile_pool(name="hpp", bufs=1, space="PSUM"))
            opp = ctx.enter_context(tc.tile_pool(name="opp", bufs=1, space="PSUM"))

            # weights go on the Act DGE queue so the SP queue's head can
            # feed tile-0 x immediately
            r_sb = consts.tile([128, 128], bf16)
            nc.scalar.dma_start(r_sb, r_d)
            w1_sb = consts.tile([128, KC, H], bf16)
            nc.scalar.dma_start(w1_sb, w1_d.rearrange("k p h -> p k h"))
            bh_sb = consts.tile([128, 1], f32)
            nc.scalar.dma_start(bh_sb, bh_d)
            fcw_sb = consts.tile([128, OUTP], bf16)
            nc.scalar.dma_start(fcw_sb, fcw_d)

            # single h bank; stage 2 waits for relu's read each
            # iteration, which runs first in the Act stream
            h_ps = hpp.tile([128, HH], f32)

            # HAM warm-up: 4 dummy back-to-back matmuls during the DMA
            # preamble shadow start releasing the PE clock gate (1.2 ->
            # 2.4 GHz); the first real (cold) tiles continue the heating,
            # so the burst only needs to bridge until tile-0 compute
            # starts, not supply the full ~3.4us itself
            warm_sb = consts.tile([128, 512], bf16, name="warm")
            nc.vector.memset(warm_sb, 0.0)
            warm_ps = opp.tile([128, 2, 512], f32, name="oA", bufs=1)
            for _ in range(4):
                nc.tensor.matmul(warm_ps[:, 0, :], lhsT=warm_sb[:, 0:128],
                                 rhs=warm_sb, skip_group_check=True)

            # x: row l of macro t lives at sub-tile l//128, partition
            # l%128; loaded TWO tiles per dma_start to halve the per-DMA
            # fixed cost (HBM completion-receipt latency)
            xv2 = x_d.rearrange("(u two s p) c -> u p two s c",
                                two=2, p=128, s=NS)
            # out: row l = 256*X + 2p + s  ->  o_sb[p, X, s, :]
            ov = o_d.rearrange("(t x p s) o -> t p x s o", p=128, s=2, x=2)

            xq, t1q, k0q, k1q, hbq, oq = {}, {}, {}, {}, {}, {}
            for it in range(nt + 3):
                j, f, g = it - 1, it - 2, it - 3

                if it < nt:
                    t1k0_ps = t1pp.tile([128, TR], f32, name="t1k0", bufs=2)
                    # stage 1 (k=0 half): t1T[c, n] = x[:,s,c0].T @ (I8 (x) lapT)
                    if it % 2 == 0:
                        x2_sb = xp.tile([128, 2, NS, C], bf16)
                        nc.sync.dma_start(x2_sb, xv2[it // 2])
                        xq[it] = xq[it + 1] = x2_sb
                    x_sb = xq[it][:, it % 2]
                    for s in range(NS):
                        nc.tensor.matmul(
                            t1k0_ps[:, s * 128:(s + 1) * 128],
                            lhsT=x_sb[:, s, 0:128],
                            rhs=r_sb,
                        )
                    k0q[it] = t1k0_ps

                if 0 <= g < nt:
                    # stage 3: row-packed pairs; pair s -> 2-bank pool
                    # tile oA (s=0) / oB (s=1), chunk X.  Separate pool
                    # tiles so next tile's pair-A matmuls wait only on
                    # the (early) Act pair-copy, not the late DVE single.
                    hb = hbq.pop(g)
                    hT_v = hb.rearrange("p (n s) -> p s n", s=2)
                    oA_ps = opp.tile([128, 2, 512], f32, name="oA", bufs=1)
                    oB_ps = opp.tile([128, 2, 512], f32, name="oB", bufs=1)
                    for s, ops in ((0, oA_ps), (1, oB_ps)):
                        nc.tensor.matmul(ops[:, 0, 0:OUTP],
                                         lhsT=hT_v[0:64, s, :],
                                         rhs=fcw_sb[0:64, :])
                        nc.tensor.matmul(ops[:, 1, 0:OUTP],
                                         lhsT=hT_v[64:128, s, :],
                                         rhs=fcw_sb[64:128, :])
                    oq[g] = (oA_ps, oB_ps)

                if 0 <= f < nt:
                    # relu+bias on DVE (max(h + bias, 0)), first in its
                    # stream: the h bank is drained before this
                    # iteration's s2 writes it, and the Act stream stays
                    # all-Identity (no activation-table switching)
                    hb = hp.tile([128, HH], bf16)
                    nc.vector.tensor_scalar(hb, h_ps, bh_sb, 0.0,
                                            mybir.AluOpType.add,
                                            mybir.AluOpType.max)
                    hbq[f] = hb

                if 0 <= j < nt:
                    # t1 casts PSUM f32 -> SBUF bf16 on DVE.  Separate
                    # k0/k1 ops: s2's first pair unblocks on k0 alone.
                    t1_sb = t1sp.tile([128, KC, TR], bf16)
                    nc.vector.tensor_copy(t1_sb[:, 0, :], k0q.pop(j))
                    nc.vector.tensor_copy(t1_sb[:, 1, :], k1q.pop(j))
                    t1q[j] = t1_sb

                if 0 <= g < nt:
                    # out copies PSUM f32 -> SBUF bf16: pair A as one Act
                    # pair-copy, pair B split: chunk 0 on Act, 1 on DVE
                    oA_ps, oB_ps = oq.pop(g)
                    o_sb = op.tile([128, 2, 2, OUT], bf16)
                    nc.scalar.copy(o_sb[:, :, 0, :], oA_ps[:, :, 0:OUT])
                    nc.scalar.copy(o_sb[:, 0, 1, :], oB_ps[:, 0, 0:OUT])
                    nc.scalar.copy(o_sb[:, 1, 1, :], oB_ps[:, 1, 0:OUT])
                    # store via the (otherwise idle) gpsimd SWDGE queue:
                    # sharing the sync HWDGE queue with the x-loads lets
                    # the store's semaphore wait head-of-line block the
                    # next x prefetch, starving stage 1
                    nc.gpsimd.dma_start(ov[g], o_sb)

                if 0 <= j < nt:
                    # stage 2, col-packed n-halves.  The group is issued
                    # TWICE (identical result; start=True replays the
                    # accumulation) -- pure PE heat to hold the duty
                    # cycle above PE_HAM's re-throttle threshold without
                    # adding any cross-engine dependency.
                    t1_sb = t1q.pop(j)
                    for k in list(range(KC)) * 2:
                        nc.tensor.matmul(
                            h_ps[0:64, :],
                            lhsT=w1_sb[:, k, :],
                            rhs=t1_sb[:, k, 0:HH],
                            start=(k == 0),
                            stop=(k == KC - 1),
                            skip_group_check=True,
                        )
                        nc.tensor.matmul(
                            h_ps[64:128, :],
                            lhsT=w1_sb[:, k, :],
                            rhs=t1_sb[:, k, HH:TR],
                            start=(k == 0),
                            stop=(k == KC - 1),
                            skip_group_check=True,
                        )

                if it < nt:
                    # stage 1 (k=1 half) last; its cast (2nd DVE op next
                    # iteration) finishes just before the next k1 write
                    x_sb = xq.pop(it)[:, it % 2]
                    t1k1_ps = t1pp.tile([128, TR], f32, name="t1k1", bufs=1)
                    for s in range(NS):
                        nc.tensor.matmul(
                            t1k1_ps[:, s * 128:(s + 1) * 128],
                            lhsT=x_sb[:, s, 128:256],
                            rhs=r_sb,
                        )
                    k1q[it] = t1k1_ps

    nc.finalize()
    return nc


def _host_prep(adj, adj_bias, cheb_w, brelu_bias, fc_w, fc_b):
    import ml_dtypes

    bf = ml_dtypes.bfloat16
    adj = np.asarray(adj, np.float32)
    w = np.maximum(adj + np.float32(adj_bias.reshape(())), 0.0)
    d = 1.0 / np.sqrt(w.sum(axis=1) + np.float32(1e-5))
    lap = np.eye(E, dtype=np.float32) - d[:, None] * w * d[None, :]

    # r = I_8 (x) lap^T : [p = b*16+j, n = b*16+i] -> lap[i, j]
    r = np.kron(np.eye(128 // E, dtype=np.float32), lap.T)

    cheb_w = np.asarray(cheb_w, np.float32)
    w1 = np.ascontiguousarray(cheb_w[1::2, :]).reshape(KC, 128, H)
    bias_h = (cheb_w[0::2, :].sum(axis=0)
              + np.asarray(brelu_bias, np.float32).reshape(H))
    bh2 = np.concatenate([bias_h, bias_h]).reshape(128, 1)
    fcw = np.zeros((128, OUTP), np.float32)
    fcw[0:H, :OUT] = np.asarray(fc_w, np.float32).T
    fcw[H:128, :OUT] = np.asarray(fc_w, np.float32).T
    return {
        "r": r.astype(bf),
        "w1": np.ascontiguousarray(w1).astype(bf),
        "bh": bh2.astype(np.float32),
        "fcw": fcw.astype(bf),
    }


def _run(inputs, trace=False, nt=NT, **kw):
    import ml_dtypes
    from concourse import bass_utils

    if nt not in _cache:
        _cache[nt] = _build_module(nt=nt)
    nc = _cache[nt]

    x = np.asarray(inputs["x"], np.float32).astype(ml_dtypes.bfloat16)
    weights = _host_prep(inputs["adj"], inputs["adj_bias"], inputs["cheb_w"],
                         inputs["brelu_bias"], inputs["fc_w"], inputs["fc_b"])

    rows = nt * TR
    shards = x.reshape(NCORES, ROWS, C)[:, :rows]
    in_maps = [dict(weights, x=np.ascontiguousarray(shards[c]))
               for c in range(NCORES)]

    res = bass_utils.run_bass_kernel_spmd(
        nc, in_maps, core_ids=list(range(NCORES)), trace=trace, **kw)

    fc_b = np.asarray(inputs["fc_b"], np.float32)
    out = np.concatenate(
        [np.asarray(res.results[c]["o"], dtype=np.float32)
           .reshape(rows // E, E, OUT)
         for c in range(NCORES)], axis=0)
    out += fc_b
    return out, res


def kernel(**inputs) -> np.ndarray:
    out, _ = _run(inputs, trace=False)
    return out
